# revision 9
# baseline (speedup 1.0000x reference)
"""Causal self-attention (B=2, T=2048, C=2048, H=16, D=128) on 8 trn2 cores.

Sharding: tensor-parallel over heads x data-parallel over batch.
Core c handles batch c//4, heads [4*(c%4) .. 4*(c%4)+4). Each core computes
qkv projection for its 4 heads, RoPE, causal attention, and a partial
output projection (its heads' rows of W_proj); the host sums the 4 partials
per batch (in fp32; device partials are fp16).

v2 design vs the DRAM-scratch baseline:
  * Everything fp16: matmuls run at full PE rate (like bf16) but with
    ~10x better mantissa than bf16; FWL (fast weight load) applies to
    non-fp32 stationary operands, hiding LDWEIGHTS (~90us exposed in the
    fp32r baseline trace); DVE ops hit the 2x packed mode; DMA halves.
  * Q^T/K^T/V stay SBUF-resident -- no DRAM scratch round trip.
  * Softmax denominator: exp blocks are accumulated into sumP on the DVE
    (fp16, 2x mode); ONE ones-matmul per (head, q-block) contracts the
    partition axis, instead of one matmul per k-block (PE -25us).
  * Diagonal S/PV matmuls shortened: block kb of q-block qb only covers
    q >= kb*128 (N in {128,256,384,512}); mask is multiplicative on P
    after exp (exp can't overflow: scores are O(5)).
  * Phase interleaving: attention wave for t-block tb is emitted with the
    QKV-projection chains of tb+1 (or phase-3 proj tiles, for the last
    wave) woven between its S/PV matmuls, so the ACT exp latency never
    stalls the PE FIFO.

Orientation (all zero-transpose):
  Q^T[d,t] = Wq^T x^T  (RoPE fused on evacuation)   K^T likewise.
  V[t,d]   = x Wv      (natural; lhsT = x^T chunk)
  S^T[k,q] = K^T.T @ Q^T ; P^T = exp(S^T/sqrt(D)) * mask
  O^T[d,q] = V.T @ P^T  (PSUM-accumulated over k-blocks)
  dn[q]    = ones.T @ sumP ; out2T = O^T * recip(dn)
  out[t,c] = sum_hd out2T_hd.T @ Wp_hd
"""

import contextlib
import math
import os
from collections import deque

import numpy as np

B, T, C = 2, 2048, 2048
H, D = 16, 128
HPC = 4  # heads per core
NCORES = 8
KT = C // 128  # 16 contraction tiles
NTB = T // 512  # 4 t-blocks

_CACHE = {}


def _build_program():
    import concourse.tile as tile
    from concourse import bacc, mybir

    f16 = mybir.dt.float16
    f32 = mybir.dt.float32
    Exp = mybir.ActivationFunctionType.Exp
    SCALE = 1.0 / math.sqrt(float(D))

    nc = bacc.Bacc(
        "TRN2", target_bir_lowering=False, debug=False, num_devices=NCORES
    )

    xT = nc.dram_tensor("xT", [C, T], f16, kind="ExternalInput").ap()
    # [p, m, k, c]: m = 8 output M-tiles (4 q heads then 4 k heads)
    wqk = nc.dram_tensor(
        "wqk", [128, 8, KT, 128], f16, kind="ExternalInput"
    ).ap()
    wv = nc.dram_tensor("wv", [128, KT, HPC * D], f16, kind="ExternalInput").ap()
    wp = nc.dram_tensor("wp", [128, HPC, C], f16, kind="ExternalInput").ap()
    cosF = nc.dram_tensor("cosF", [128, T], f16, kind="ExternalInput").ap()
    sinF = nc.dram_tensor("sinF", [128, T], f16, kind="ExternalInput").ap()
    onesd = nc.dram_tensor("onesd", [128, 128], f16, kind="ExternalInput").ap()
    maskd = nc.dram_tensor("maskd", [128, 128], f16, kind="ExternalInput").ap()
    out = nc.dram_tensor("out", [T, C], f16, kind="ExternalOutput").ap()

    with tile.TileContext(nc) as tc:
        with (
            tc.tile_pool(name="consts", bufs=1) as consts,
            tc.tile_pool(name="qkv", bufs=1) as qkvp,
            tc.tile_pool(name="pt", bufs=6) as ptp,
            tc.tile_pool(name="sump", bufs=2) as sumpp,
            tc.tile_pool(name="rb", bufs=2) as rbp,
            tc.tile_pool(name="ob", bufs=4) as obp,
            tc.tile_pool(name="stps", bufs=2, space="PSUM") as stps,
            tc.tile_pool(name="pvps", bufs=2, space="PSUM") as pvps,
            tc.tile_pool(name="dnps", bufs=1, space="PSUM") as dnpsp,
        ):
            es = contextlib.ExitStack()
            p1x = es.enter_context(tc.tile_pool(name="p1x", bufs=2))
            p1w = es.enter_context(tc.tile_pool(name="p1w", bufs=1))
            p1e = es.enter_context(tc.tile_pool(name="p1e", bufs=2))
            p1ps = es.enter_context(
                tc.tile_pool(name="p1ps", bufs=2, space="PSUM")
            )

            # ---------------- persistent SBUF ----------------
            cos_sb = consts.tile([128, T], f16, tag="cos")
            sin_sb = consts.tile([128, T], f16, tag="sin")
            ones_sb = consts.tile([128, 128], f16, tag="ones")
            mask_sb = consts.tile([128, 128], f16, tag="mask")
            wp_sb = consts.tile([128, HPC, C], f16, tag="wp")
            wqkg = p1w.tile([128, 8, KT, 128], f16, tag="wqkg")
            wv_sb = p1w.tile([128, KT, HPC * D], f16, tag="wv")
            qkT = qkvp.tile([128, 8, T], f16, tag="qkT")
            vt = qkvp.tile([128, KT, HPC * D], f16, tag="vt")
            out2T = qkvp.tile([128, HPC, T], f16, tag="out2T")

            # ---------------- helpers ----------------
            def emit_xtb_dma(tb):
                xtb = p1x.tile([128, KT, 512], f16, tag="xtb", name=f"xtb{tb}")
                for k in range(KT):
                    nc.sync.dma_start(
                        out=xtb[:, k],
                        in_=xT[k * 128 : (k + 1) * 128,
                              tb * 512 : (tb + 1) * 512],
                    )
                return xtb

            def rope_evac(ps, m, tb):
                # cross-partition reads must come from PSUM (SB+SB operands
                # are required to share a base partition)
                tsl = slice(tb * 512, (tb + 1) * 512)
                qraw = p1e.tile([128, 512], f16, tag="qraw")
                nc.scalar.copy(qraw, ps)
                t1 = p1e.tile([128, 512], f16, tag="t1")
                nc.vector.tensor_mul(t1[0:64], ps[64:128], sin_sb[0:64, tsl])
                nc.vector.tensor_mul(t1[64:128], ps[0:64],
                                     sin_sb[64:128, tsl])
                gq = p1e.tile([128, 512], f16, tag="gq")
                nc.vector.tensor_mul(gq, qraw, cos_sb[:, tsl])
                nc.vector.tensor_add(qkT[:, m, tsl], gq, t1)

            def p1_steps(tb, xtb):
                """QKV projection for t-block tb; yields every 2 matmuls."""
                for m in range(8):
                    ps = p1ps.tile([128, 512], f32, tag="qk")
                    for k in range(KT):
                        nc.tensor.matmul(
                            ps,
                            lhsT=wqkg[:, m, k, :],
                            rhs=xtb[:, k, :],
                            start=(k == 0),
                            stop=(k == KT - 1),
                        )
                        if k % 2 == 1:
                            if k == KT - 1:
                                rope_evac(ps, m, tb)
                            yield
                for t4 in range(4):
                    psv = p1ps.tile([128, 512], f32, tag="v", bufs=1)
                    for k in range(KT):
                        nc.tensor.matmul(
                            psv,
                            lhsT=xtb[:, k, t4 * 128 : (t4 + 1) * 128],
                            rhs=wv_sb[:, k, :],
                            start=(k == 0),
                            stop=(k == KT - 1),
                        )
                        if k % 2 == 1:
                            if k == KT - 1:
                                nc.scalar.copy(vt[:, tb * 4 + t4, :], psv)
                            yield

            def p3_steps(trange, posp):
                """Output projection tiles; yields every 2 matmuls."""
                for t in trange:
                    tsl = slice(t * 128, (t + 1) * 128)
                    for cb in range(4):
                        csl = slice(cb * 512, (cb + 1) * 512)
                        pos = posp.tile([128, 512], f32, tag="pos")
                        for hd in range(HPC):
                            nc.tensor.matmul(
                                pos,
                                lhsT=out2T[:, hd, tsl],
                                rhs=wp_sb[:, hd, csl],
                                start=(hd == 0),
                                stop=(hd == HPC - 1),
                            )
                            if hd % 2 == 1:
                                if hd == HPC - 1:
                                    ob = obp.tile([128, 512], f16, tag="ob")
                                    nc.vector.tensor_copy(ob, pos)
                                    nc.scalar.dma_start(
                                        out=out[tsl, csl], in_=ob
                                    )
                                yield

            def emit_pv(pv, h, nk, pt, qoff, kb):
                nc.tensor.matmul(
                    pv[:, qoff:],
                    lhsT=vt[:, kb, h * 128 : (h + 1) * 128],
                    rhs=pt[:, qoff:],
                    start=(kb == 0),
                    stop=(kb == nk - 1),
                )

            def emit_wave(tb, filler):
                qsl = slice(tb * 512, (tb + 1) * 512)
                nk = 4 * (tb + 1)
                for h in range(HPC):
                    sumP = sumpp.tile([128, 512], f16, tag="sumP")
                    pv = pvps.tile([128, 512], f32, tag="pv")
                    pending = deque()
                    for kb in range(nk):
                        j = kb - 4 * tb
                        qoff = max(0, j * 128)
                        st = stps.tile([128, 512], f32, tag="st")
                        nc.tensor.matmul(
                            st[:, qoff:],
                            lhsT=qkT[:, 4 + h, kb * 128 : (kb + 1) * 128],
                            rhs=qkT[:, h, tb * 512 + qoff : (tb + 1) * 512],
                            start=True,
                            stop=True,
                        )
                        pt = ptp.tile([128, 512], f16, tag="pt")
                        nc.scalar.activation(
                            pt[:, qoff:], st[:, qoff:], Exp, scale=SCALE
                        )
                        if j >= 0:
                            nc.vector.tensor_mul(
                                pt[:, qoff : qoff + 128],
                                pt[:, qoff : qoff + 128],
                                mask_sb,
                            )
                        if kb == 0:
                            nc.vector.tensor_copy(sumP, pt)
                        else:
                            nc.vector.tensor_add(
                                sumP[:, qoff:], sumP[:, qoff:], pt[:, qoff:]
                            )
                        next(filler, None)
                        pending.append((pt, qoff, kb))
                        if len(pending) >= 3:
                            emit_pv(pv, h, nk, *pending.popleft())
                    while pending:
                        emit_pv(pv, h, nk, *pending.popleft())
                    dn = dnpsp.tile([128, 512], f32, tag="dn")
                    nc.tensor.matmul(
                        dn, lhsT=ones_sb, rhs=sumP, start=True, stop=True
                    )
                    rb = rbp.tile([128, 512], f32, tag="rb")
                    nc.vector.reciprocal_approx_fast(out=rb, in_=dn)
                    nc.vector.tensor_mul(out2T[:, h, qsl], pv, rb)

            # ---------------- DMA preload ----------------
            # warm the ACT spline table set (exp) during the initial DMA wait
            dummy = p1e.tile([128, 8], f16, tag="dummy", bufs=1, name="dummy")
            nc.vector.memset(dummy, 0.0)
            nc.scalar.activation(dummy, dummy, Exp, scale=1.0)
            # warm the PE HAM clock gate (cold 1.2GHz -> 2.4GHz needs ~3.4us
            # of sustained matmul activity) with dummy matmuls on zeroed SBUF
            # while the first weight/x DMAs stream in; PE is idle regardless.
            warm = p1e.tile([128, 512], f16, tag="warm", bufs=1, name="warm")
            nc.vector.memset(warm, 0.0)
            warmps = dnpsp.tile([128, 512], f32, tag="dn", name="warmps")
            for i in range(24):
                nc.tensor.matmul(
                    warmps, lhsT=warm[:, 0:128], rhs=warm,
                    start=True, stop=True,
                )
            # first matmul chain (m=0) needs only (wqk[0,k], x[k]) pairs:
            # interleave k-granular loads so PE starts after ~64KB
            xtb0 = p1x.tile([128, KT, 512], f16, tag="xtb", name="xtb0")
            for k in range(KT):
                nc.sync.dma_start(out=wqkg[:, 0, k], in_=wqk[:, 0, k])
                nc.sync.dma_start(
                    out=xtb0[:, k], in_=xT[k * 128 : (k + 1) * 128, 0:512]
                )
            for m in range(1, 8):
                nc.sync.dma_start(out=wqkg[:, m], in_=wqk[:, m])
                if m == 1:
                    nc.sync.dma_start(out=cos_sb, in_=cosF)
                    nc.sync.dma_start(out=sin_sb, in_=sinF)
            for k in range(KT):
                nc.sync.dma_start(out=wv_sb[:, k], in_=wv[:, k])
            nc.sync.dma_start(out=ones_sb, in_=onesd)
            nc.sync.dma_start(out=mask_sb, in_=maskd)
            nc.sync.dma_start(out=wp_sb, in_=wp)

            # ---------------- emission ----------------
            for _ in p1_steps(0, xtb0):
                pass
            xtb_n = emit_xtb_dma(1)
            filler = p1_steps(1, xtb_n)
            for tb in range(NTB):
                if tb == NTB - 1:
                    # all qkv projections done: swap PSUM to phase 3
                    es.close()
                    es3 = contextlib.ExitStack()
                    posp = es3.enter_context(
                        tc.tile_pool(name="posps", bufs=2, space="PSUM")
                    )
                    filler = p3_steps(range(12), posp)
                emit_wave(tb, filler)
                for _ in filler:
                    pass
                if tb + 2 < NTB:
                    xtb_n = emit_xtb_dma(tb + 2)
                    filler = p1_steps(tb + 2, xtb_n)
            for _ in p3_steps(range(12, 16), posp):
                pass
            es3.close()
    nc.compile()
    return nc


def _get_program():
    if "nc" not in _CACHE:
        _CACHE["nc"] = _build_program()
    return _CACHE["nc"]


def make_in_maps(x, cos, sin, W_qkv, W_proj):
    """Host-side sharding: per-core input dicts (numpy, fp16)."""
    f16 = np.float16
    x = np.asarray(x, dtype=np.float32)
    cos = np.asarray(cos, dtype=np.float32)
    sin = np.asarray(sin, dtype=np.float32)
    W_qkv = np.asarray(W_qkv, dtype=np.float32)
    W_proj = np.asarray(W_proj, dtype=np.float32)

    cosF = np.ascontiguousarray(np.tile(cos.T, (2, 1)).astype(f16))
    sinF = np.ascontiguousarray(
        np.concatenate([-sin.T, sin.T], axis=0).astype(f16)
    )
    kl = np.arange(128)[:, None]
    ql = np.arange(128)[None, :]
    mask01 = (ql >= kl).astype(f16)  # [128k, 128q] multiplicative
    ones = np.ones((128, 128), dtype=f16)

    in_maps = []
    for core in range(NCORES):
        b, hg = core // 4, core % 4
        csl = slice(hg * 512, (hg + 1) * 512)
        wqk_cat = np.concatenate(
            [W_qkv[:, csl], W_qkv[:, C + hg * 512 : C + (hg + 1) * 512]],
            axis=1,
        )  # [C, 1024]
        wqk_np = np.ascontiguousarray(
            wqk_cat.reshape(KT, 128, 8, 128).transpose(1, 2, 0, 3).astype(f16)
        )  # [p, m, k, c]
        wv_np = np.ascontiguousarray(
            W_qkv[:, 2 * C + hg * 512 : 2 * C + (hg + 1) * 512]
            .reshape(KT, 128, HPC * D)
            .transpose(1, 0, 2)
            .astype(f16)
        )  # [p, k, c]
        wp_np = np.ascontiguousarray(
            W_proj[hg * 512 : (hg + 1) * 512, :]
            .reshape(HPC, 128, C)
            .transpose(1, 0, 2)
            .astype(f16)
        )  # [p, hd, c]
        xT_np = np.ascontiguousarray(x[b].T.astype(f16))
        in_maps.append(
            {
                "xT": xT_np,
                "wqk": wqk_np,
                "wv": wv_np,
                "wp": wp_np,
                "cosF": cosF,
                "sinF": sinF,
                "onesd": ones,
                "maskd": mask01,
            }
        )
    return in_maps


def kernel(x, cos, sin, W_qkv, W_proj):
    from concourse.bass_utils import run_bass_kernel_spmd

    nc = _get_program()
    in_maps = make_in_maps(x, cos, sin, W_qkv, W_proj)
    trace = bool(int(os.environ.get("KERNEL_TRACE", "0")))
    res = run_bass_kernel_spmd(
        nc, in_maps, core_ids=list(range(NCORES)), trace=trace
    )
    if trace:
        _CACHE["last_results"] = res
        if res.exec_time_ns is not None:
            print(f"HW exec time: {res.exec_time_ns} ns")

    out = np.zeros((B, T, C), dtype=np.float32)
    for core in range(NCORES):
        out[core // 4] += res.results[core]["out"].astype(np.float32)
    return out


# revision 11
# speedup vs baseline: 1.0071x; 1.0071x over previous
"""Causal self-attention (B=2, T=2048, C=2048, H=16, D=128) on 8 trn2 cores.

Sharding: tensor-parallel over heads x data-parallel over batch.
Core c handles batch c//4, heads [4*(c%4) .. 4*(c%4)+4). Each core computes
qkv projection for its 4 heads, RoPE, causal attention, and a partial
output projection (its heads' rows of W_proj); the host sums the 4 partials
per batch (in fp32; device partials are fp16).

v2 design vs the DRAM-scratch baseline:
  * Everything fp16: matmuls run at full PE rate (like bf16) but with
    ~10x better mantissa than bf16; FWL (fast weight load) applies to
    non-fp32 stationary operands, hiding LDWEIGHTS (~90us exposed in the
    fp32r baseline trace); DVE ops hit the 2x packed mode; DMA halves.
  * Q^T/K^T/V stay SBUF-resident -- no DRAM scratch round trip.
  * Softmax denominator: exp blocks are accumulated into sumP on the DVE
    (fp16, 2x mode); ONE ones-matmul per (head, q-block) contracts the
    partition axis, instead of one matmul per k-block (PE -25us).
  * Diagonal S/PV matmuls shortened: block kb of q-block qb only covers
    q >= kb*128 (N in {128,256,384,512}); mask is multiplicative on P
    after exp (exp can't overflow: scores are O(5)).
  * Phase interleaving: attention wave for t-block tb is emitted with the
    QKV-projection chains of tb+1 (or phase-3 proj tiles, for the last
    wave) woven between its S/PV matmuls, so the ACT exp latency never
    stalls the PE FIFO.

Orientation (all zero-transpose):
  Q^T[d,t] = Wq^T x^T  (RoPE fused on evacuation)   K^T likewise.
  V[t,d]   = x Wv      (natural; lhsT = x^T chunk)
  S^T[k,q] = K^T.T @ Q^T ; P^T = exp(S^T/sqrt(D)) * mask
  O^T[d,q] = V.T @ P^T  (PSUM-accumulated over k-blocks)
  dn[q]    = ones.T @ sumP ; out2T = O^T * recip(dn)
  out[t,c] = sum_hd out2T_hd.T @ Wp_hd
"""

import contextlib
import math
import os
from collections import deque

import numpy as np

B, T, C = 2, 2048, 2048
H, D = 16, 128
HPC = 4  # heads per core
NCORES = 8
KT = C // 128  # 16 contraction tiles
NTB = T // 512  # 4 t-blocks

_CACHE = {}


def _build_program():
    import concourse.tile as tile
    from concourse import bacc, mybir

    f16 = mybir.dt.float16
    f32 = mybir.dt.float32
    Exp = mybir.ActivationFunctionType.Exp
    SCALE = 1.0 / math.sqrt(float(D))

    nc = bacc.Bacc(
        "TRN2", target_bir_lowering=False, debug=False, num_devices=NCORES
    )

    xT = nc.dram_tensor("xT", [C, T], f16, kind="ExternalInput").ap()
    # [p, m, k, c]: m = 8 output M-tiles (4 q heads then 4 k heads)
    wqk = nc.dram_tensor(
        "wqk", [128, 8, KT, 128], f16, kind="ExternalInput"
    ).ap()
    wv = nc.dram_tensor("wv", [128, KT, HPC * D], f16, kind="ExternalInput").ap()
    wp = nc.dram_tensor("wp", [128, HPC, C], f16, kind="ExternalInput").ap()
    cosF = nc.dram_tensor("cosF", [128, T], f16, kind="ExternalInput").ap()
    sinF = nc.dram_tensor("sinF", [128, T], f16, kind="ExternalInput").ap()
    onesd = nc.dram_tensor("onesd", [128, 128], f16, kind="ExternalInput").ap()
    maskd = nc.dram_tensor("maskd", [128, 128], f16, kind="ExternalInput").ap()
    out = nc.dram_tensor("out", [T, C], f16, kind="ExternalOutput").ap()

    with tile.TileContext(nc) as tc:
        with (
            tc.tile_pool(name="consts", bufs=1) as consts,
            tc.tile_pool(name="qkv", bufs=1) as qkvp,
            tc.tile_pool(name="pt", bufs=6) as ptp,
            tc.tile_pool(name="sump", bufs=2) as sumpp,
            tc.tile_pool(name="rb", bufs=2) as rbp,
            tc.tile_pool(name="ob", bufs=4) as obp,
            tc.tile_pool(name="stps", bufs=2, space="PSUM") as stps,
            tc.tile_pool(name="pvps", bufs=2, space="PSUM") as pvps,
            tc.tile_pool(name="dnps", bufs=1, space="PSUM") as dnpsp,
        ):
            es = contextlib.ExitStack()
            p1x = es.enter_context(tc.tile_pool(name="p1x", bufs=2))
            p1w = es.enter_context(tc.tile_pool(name="p1w", bufs=1))
            p1e = es.enter_context(tc.tile_pool(name="p1e", bufs=2))
            p1ps = es.enter_context(
                tc.tile_pool(name="p1ps", bufs=2, space="PSUM")
            )

            # ---------------- persistent SBUF ----------------
            cos_sb = consts.tile([128, T], f16, tag="cos")
            sin_sb = consts.tile([128, T], f16, tag="sin")
            ones_sb = consts.tile([128, 128], f16, tag="ones")
            mask_sb = consts.tile([128, 128], f16, tag="mask")
            wp_sb = consts.tile([128, HPC, C], f16, tag="wp")
            wqkg = p1w.tile([128, 8, KT, 128], f16, tag="wqkg")
            wv_sb = p1w.tile([128, KT, HPC * D], f16, tag="wv")
            qkT = qkvp.tile([128, 8, T], f16, tag="qkT")
            vt = qkvp.tile([128, KT, HPC * D], f16, tag="vt")
            out2T = qkvp.tile([128, HPC, T], f16, tag="out2T")

            # ---------------- helpers ----------------
            def emit_xtb_dma(tb):
                xtb = p1x.tile([128, KT, 512], f16, tag="xtb", name=f"xtb{tb}")
                for k in range(KT):
                    nc.sync.dma_start(
                        out=xtb[:, k],
                        in_=xT[k * 128 : (k + 1) * 128,
                              tb * 512 : (tb + 1) * 512],
                    )
                return xtb

            def rope_evac(ps, m, tb):
                # cross-partition reads must come from PSUM (SB+SB operands
                # are required to share a base partition)
                tsl = slice(tb * 512, (tb + 1) * 512)
                qraw = p1e.tile([128, 512], f16, tag="qraw")
                nc.scalar.copy(qraw, ps)
                t1 = p1e.tile([128, 512], f16, tag="t1")
                nc.vector.tensor_mul(t1[0:64], ps[64:128], sin_sb[0:64, tsl])
                nc.vector.tensor_mul(t1[64:128], ps[0:64],
                                     sin_sb[64:128, tsl])
                gq = p1e.tile([128, 512], f16, tag="gq")
                nc.gpsimd.tensor_mul(gq, qraw, cos_sb[:, tsl])
                nc.vector.tensor_add(qkT[:, m, tsl], gq, t1)

            def p1_steps(tb, xtb):
                """QKV projection for t-block tb; yields every 2 matmuls."""
                for m in range(8):
                    ps = p1ps.tile([128, 512], f32, tag="qk")
                    for k in range(KT):
                        nc.tensor.matmul(
                            ps,
                            lhsT=wqkg[:, m, k, :],
                            rhs=xtb[:, k, :],
                            start=(k == 0),
                            stop=(k == KT - 1),
                        )
                        if k % 2 == 1:
                            if k == KT - 1:
                                rope_evac(ps, m, tb)
                            yield
                for t4 in range(4):
                    psv = p1ps.tile([128, 512], f32, tag="v", bufs=1)
                    for k in range(KT):
                        nc.tensor.matmul(
                            psv,
                            lhsT=xtb[:, k, t4 * 128 : (t4 + 1) * 128],
                            rhs=wv_sb[:, k, :],
                            start=(k == 0),
                            stop=(k == KT - 1),
                        )
                        if k % 2 == 1:
                            if k == KT - 1:
                                nc.scalar.copy(vt[:, tb * 4 + t4, :], psv)
                            yield

            def p3_steps(trange, posp):
                """Output projection tiles; yields every 2 matmuls."""
                for t in trange:
                    tsl = slice(t * 128, (t + 1) * 128)
                    for cb in range(4):
                        csl = slice(cb * 512, (cb + 1) * 512)
                        pos = posp.tile([128, 512], f32, tag="pos")
                        for hd in range(HPC):
                            nc.tensor.matmul(
                                pos,
                                lhsT=out2T[:, hd, tsl],
                                rhs=wp_sb[:, hd, csl],
                                start=(hd == 0),
                                stop=(hd == HPC - 1),
                            )
                            if hd % 2 == 1:
                                if hd == HPC - 1:
                                    ob = obp.tile([128, 512], f16, tag="ob")
                                    nc.vector.tensor_copy(ob, pos)
                                    nc.scalar.dma_start(
                                        out=out[tsl, csl], in_=ob
                                    )
                                yield

            def emit_pv(pv, h, nk, pt, qoff, kb):
                nc.tensor.matmul(
                    pv[:, qoff:],
                    lhsT=vt[:, kb, h * 128 : (h + 1) * 128],
                    rhs=pt[:, qoff:],
                    start=(kb == 0),
                    stop=(kb == nk - 1),
                )

            def emit_wave(tb, filler):
                qsl = slice(tb * 512, (tb + 1) * 512)
                nk = 4 * (tb + 1)
                for h in range(HPC):
                    sumP = sumpp.tile([128, 512], f16, tag="sumP")
                    pv = pvps.tile([128, 512], f32, tag="pv")
                    pending = deque()
                    for kb in range(nk):
                        j = kb - 4 * tb
                        qoff = max(0, j * 128)
                        st = stps.tile([128, 512], f32, tag="st")
                        nc.tensor.matmul(
                            st[:, qoff:],
                            lhsT=qkT[:, 4 + h, kb * 128 : (kb + 1) * 128],
                            rhs=qkT[:, h, tb * 512 + qoff : (tb + 1) * 512],
                            start=True,
                            stop=True,
                        )
                        pt = ptp.tile([128, 512], f16, tag="pt")
                        nc.scalar.activation(
                            pt[:, qoff:], st[:, qoff:], Exp, scale=SCALE
                        )
                        if j >= 0:
                            nc.vector.tensor_mul(
                                pt[:, qoff : qoff + 128],
                                pt[:, qoff : qoff + 128],
                                mask_sb,
                            )
                        if kb == 0:
                            nc.vector.tensor_copy(sumP, pt)
                        else:
                            nc.vector.tensor_add(
                                sumP[:, qoff:], sumP[:, qoff:], pt[:, qoff:]
                            )
                        next(filler, None)
                        pending.append((pt, qoff, kb))
                        if len(pending) >= 3:
                            emit_pv(pv, h, nk, *pending.popleft())
                    while pending:
                        emit_pv(pv, h, nk, *pending.popleft())
                    dn = dnpsp.tile([128, 512], f32, tag="dn")
                    nc.tensor.matmul(
                        dn, lhsT=ones_sb, rhs=sumP, start=True, stop=True
                    )
                    rb = rbp.tile([128, 512], f32, tag="rb")
                    nc.vector.reciprocal_approx_fast(out=rb, in_=dn)
                    nc.vector.tensor_mul(out2T[:, h, qsl], pv, rb)

            # ---------------- DMA preload ----------------
            # warm the ACT spline table set (exp) during the initial DMA wait
            dummy = p1e.tile([128, 8], f16, tag="dummy", bufs=1, name="dummy")
            nc.vector.memset(dummy, 0.0)
            nc.scalar.activation(dummy, dummy, Exp, scale=1.0)
            # warm the PE HAM clock gate (cold 1.2GHz -> 2.4GHz needs ~3.4us
            # of sustained matmul activity) with dummy matmuls on zeroed SBUF
            # while the first weight/x DMAs stream in; PE is idle regardless.
            warm = p1e.tile([128, 512], f16, tag="warm", bufs=1, name="warm")
            nc.vector.memset(warm, 0.0)
            warmps = dnpsp.tile([128, 512], f32, tag="dn", name="warmps")
            for i in range(10):
                nc.tensor.matmul(
                    warmps, lhsT=warm[:, 0:128], rhs=warm,
                    start=True, stop=True,
                )
            # first matmul chain (m=0) needs only (wqk[0,k], x[k]) pairs:
            # interleave k-granular loads so PE starts after ~64KB
            xtb0 = p1x.tile([128, KT, 512], f16, tag="xtb", name="xtb0")
            for k in range(KT):
                nc.sync.dma_start(out=wqkg[:, 0, k], in_=wqk[:, 0, k])
                nc.sync.dma_start(
                    out=xtb0[:, k], in_=xT[k * 128 : (k + 1) * 128, 0:512]
                )
            for m in range(1, 8):
                nc.sync.dma_start(out=wqkg[:, m], in_=wqk[:, m])
                if m == 1:
                    nc.sync.dma_start(out=cos_sb, in_=cosF)
                    nc.sync.dma_start(out=sin_sb, in_=sinF)
            for k in range(KT):
                nc.sync.dma_start(out=wv_sb[:, k], in_=wv[:, k])
            nc.sync.dma_start(out=ones_sb, in_=onesd)
            nc.sync.dma_start(out=mask_sb, in_=maskd)
            nc.sync.dma_start(out=wp_sb, in_=wp)

            # ---------------- emission ----------------
            for _ in p1_steps(0, xtb0):
                pass
            xtb_n = emit_xtb_dma(1)
            filler = p1_steps(1, xtb_n)
            for tb in range(NTB):
                if tb == NTB - 1:
                    # all qkv projections done: swap PSUM to phase 3
                    es.close()
                    es3 = contextlib.ExitStack()
                    posp = es3.enter_context(
                        tc.tile_pool(name="posps", bufs=2, space="PSUM")
                    )
                    filler = p3_steps(range(12), posp)
                emit_wave(tb, filler)
                for _ in filler:
                    pass
                if tb + 2 < NTB:
                    xtb_n = emit_xtb_dma(tb + 2)
                    filler = p1_steps(tb + 2, xtb_n)
            for _ in p3_steps(range(12, 16), posp):
                pass
            es3.close()
    nc.compile()
    return nc


def _get_program():
    if "nc" not in _CACHE:
        _CACHE["nc"] = _build_program()
    return _CACHE["nc"]


def make_in_maps(x, cos, sin, W_qkv, W_proj):
    """Host-side sharding: per-core input dicts (numpy, fp16)."""
    f16 = np.float16
    x = np.asarray(x, dtype=np.float32)
    cos = np.asarray(cos, dtype=np.float32)
    sin = np.asarray(sin, dtype=np.float32)
    W_qkv = np.asarray(W_qkv, dtype=np.float32)
    W_proj = np.asarray(W_proj, dtype=np.float32)

    cosF = np.ascontiguousarray(np.tile(cos.T, (2, 1)).astype(f16))
    sinF = np.ascontiguousarray(
        np.concatenate([-sin.T, sin.T], axis=0).astype(f16)
    )
    kl = np.arange(128)[:, None]
    ql = np.arange(128)[None, :]
    mask01 = (ql >= kl).astype(f16)  # [128k, 128q] multiplicative
    ones = np.ones((128, 128), dtype=f16)

    in_maps = []
    for core in range(NCORES):
        b, hg = core // 4, core % 4
        csl = slice(hg * 512, (hg + 1) * 512)
        wqk_cat = np.concatenate(
            [W_qkv[:, csl], W_qkv[:, C + hg * 512 : C + (hg + 1) * 512]],
            axis=1,
        )  # [C, 1024]
        wqk_np = np.ascontiguousarray(
            wqk_cat.reshape(KT, 128, 8, 128).transpose(1, 2, 0, 3).astype(f16)
        )  # [p, m, k, c]
        wv_np = np.ascontiguousarray(
            W_qkv[:, 2 * C + hg * 512 : 2 * C + (hg + 1) * 512]
            .reshape(KT, 128, HPC * D)
            .transpose(1, 0, 2)
            .astype(f16)
        )  # [p, k, c]
        wp_np = np.ascontiguousarray(
            W_proj[hg * 512 : (hg + 1) * 512, :]
            .reshape(HPC, 128, C)
            .transpose(1, 0, 2)
            .astype(f16)
        )  # [p, hd, c]
        xT_np = np.ascontiguousarray(x[b].T.astype(f16))
        in_maps.append(
            {
                "xT": xT_np,
                "wqk": wqk_np,
                "wv": wv_np,
                "wp": wp_np,
                "cosF": cosF,
                "sinF": sinF,
                "onesd": ones,
                "maskd": mask01,
            }
        )
    return in_maps


def kernel(x, cos, sin, W_qkv, W_proj):
    from concourse.bass_utils import run_bass_kernel_spmd

    nc = _get_program()
    in_maps = make_in_maps(x, cos, sin, W_qkv, W_proj)
    trace = bool(int(os.environ.get("KERNEL_TRACE", "0")))
    res = run_bass_kernel_spmd(
        nc, in_maps, core_ids=list(range(NCORES)), trace=trace
    )
    if trace:
        _CACHE["last_results"] = res
        if res.exec_time_ns is not None:
            print(f"HW exec time: {res.exec_time_ns} ns")

    out = np.zeros((B, T, C), dtype=np.float32)
    for core in range(NCORES):
        out[core // 4] += res.results[core]["out"].astype(np.float32)
    return out
